# revision 17
# baseline (speedup 1.0000x reference)
"""RNN-T JointNetwork kernel for 8 Trainium2 NeuronCores.

Math: out[b,t,u,:] = tanh(concat(fe[b,t], gd[b,u])) @ Wj + bj
with fe = f@We+be, gd = g@Wd+bd.

Since tanh acts elementwise and the concat feeds a single GEMM, the joint
GEMM factorizes exactly:
    out[b,t,u,:] = A[b,t,:] + C[b,u,:]
    A = tanh(f@We+be) @ Wj[:Dm]          (per-(b,t) row)
    C = tanh(g@Wd+bd) @ Wj[Dm:] + bj     (per-(b,u) row)
This collapses the 137-GFLOP joint GEMM into two tiny GEMMs plus a
broadcast-add, leaving the kernel bound by the output write.

Sharding: 8 cores, core c owns (b = c//2, t-half = c%2) -> a [128,64,V]
output chunk per core.

Measured facts this design is built around (profiled on trn2):
  - each dma_start costs ~650 ns serial issue time on the Sync sequencer
    -> pack inputs into 9 DMAs (concurrent transfers aggregate to
    ~400 GB/s; serializing them into stages measured SLOWER, a lone DMA
    only sustains ~270 GB/s, so they are just issued back-to-back)
  - gpsimd affine_select is ~5 us per op -> selector constants are
    shipped from the host inside the input stream (fp8: 0/1 values are
    exact and mixed fp8-weights x bf16-rhs matmul works)
  - DMA write bandwidth ~400 GB/s only with large contiguous runs ->
    output rows are permuted so each partition writes 4 consecutive
    DRAM rows = one 8 KB descriptor (out tensor is bf16: tolerance 2e-2
    dwarfs bf16's ~5e-3; host upcasts to fp32)
  - fp32 PSUM->SBUF moves run at 1x on DVE and ACT alike -> alternate
    the per-tile output moves across both engines
  - PE HAM clock gate starts at 1.2 GHz; ~20 dummy matmuls during the
    input DMA window warm it to 2.4 GHz before the real GEMMs

On-core plan (bf16 everywhere, fp32 only in PSUM):
  - tfT[m,t] = tanh(We.T@fT + be), tgT likewise (PE bf16 + ACT tanh)
  - ACp0 = [A(0:64) ; C] and ACp1 = [C ; A(64:128)] packed bf16 [128,V];
    C is computed once into psum rows 64:128 (+bj via K=1 ones matmul),
    copied to ACp0, then replicated to ACp1's rows 0:64 with a K=64
    identity matmul (cheaper than re-running the K=512 GEMM)
  - output chunk j covers out rows 512j..512j+512; psum tile a holds
    rows 4p+a so partition p's SBUF bytes map to 4 consecutive DRAM
    rows; ONE K=128 selector matmul per 512-col bank picks the A row
    and C row and sums them in fp32 PSUM
"""

import sys

sys.path.insert(0, "/opt/trn_rl_repo")

import numpy as np

import concourse.bacc as bacc
import concourse.mybir as mybir
import concourse.tile as tile
from concourse.bass_utils import run_bass_kernel_spmd

B, T, U = 4, 256, 64
D = 512  # DE = DD = DM
V = 1024
TC = 128  # t rows per core
NCORES = 8
FP32 = mybir.dt.float32
BF16 = mybir.dt.bfloat16
NPBF16 = mybir.dt.np(mybir.dt.bfloat16)
FP8 = mybir.dt.float8e4
NPFP8 = mybir.dt.np(mybir.dt.float8e4)
TANH = mybir.ActivationFunctionType.Tanh

# pack1 column offsets (per-core tensor: fT | We | gT | Wd | swap-ident)
OFF_FT, OFF_WE, OFF_GT, OFF_WD, OFF_SW = 0, 512, 2560, 2816, 4864
PACK1_COLS = 4928

_cache = {}


def _build_nc():
    nc = bacc.Bacc("TRN2", target_bir_lowering=False)

    pack1_d = nc.dram_tensor("pack1", [128, PACK1_COLS], BF16, kind="ExternalInput")
    wjt_d = [nc.dram_tensor(f"wjt{i}", [128, 2048], BF16, kind="ExternalInput") for i in range(2)]
    wjb_d = [nc.dram_tensor(f"wjb{i}", [128, 2048], BF16, kind="ExternalInput") for i in range(2)]
    sel_d = [nc.dram_tensor(f"sel{i}", [128, 4096], FP8, kind="ExternalInput") for i in range(2)]
    bpack_d = nc.dram_tensor("bpack", [128, 8], FP32, kind="ExternalInput")
    bjp_d = nc.dram_tensor("bjp", [1, V + 128], BF16, kind="ExternalInput")
    out_d = nc.dram_tensor("out", [16 * 128, 4 * V], BF16, kind="ExternalOutput")

    with tile.TileContext(nc) as tc:
        with tc.tile_pool(name="wts", bufs=1) as wp:
            pack1 = wp.tile([128, PACK1_COLS], BF16, tag="pack1")
            wjt = [wp.tile([128, 2048], BF16, tag=f"wjt{i}", name=f"wjt{i}") for i in range(2)]
            wjb = [wp.tile([128, 2048], BF16, tag=f"wjb{i}", name=f"wjb{i}") for i in range(2)]
            sel = [wp.tile([128, 4096], FP8, tag=f"sel{i}", name=f"sel{i}") for i in range(2)]
            bpack = wp.tile([128, 8], FP32, tag="bpack")
            bjp = wp.tile([1, V + 128], BF16, tag="bjp")
            tfT = [wp.tile([128, TC], BF16, tag=f"tfT{c}", name=f"tfT{c}") for c in range(4)]
            tgT = [wp.tile([128, U], BF16, tag=f"tgT{c}", name=f"tgT{c}") for c in range(4)]
            ACp0 = wp.tile([128, V], BF16, tag="ACp0")
            ACp1 = wp.tile([128, V], BF16, tag="ACp1")

            # input stream: issued back-to-back; transfers interleave at
            # packet level across queues (measured: concurrent DMAs
            # aggregate to ~400 GB/s, a lone DMA only ~270), so no
            # explicit ordering -- everything lands by ~22 us and the
            # prologue GEMMs interleave with the arrivals
            nc.sync.dma_start(pack1[:], pack1_d[:])
            nc.sync.dma_start(bpack[:], bpack_d[:])
            nc.sync.dma_start(bjp[:], bjp_d[:])
            for i in range(2):
                nc.sync.dma_start(wjb[i][:], wjb_d[i][:])
            for i in range(2):
                nc.sync.dma_start(wjt[i][:], wjt_d[i][:])
            nc.sync.dma_start(sel[0][:], sel_d[0][:])
            nc.sync.dma_start(sel[1][:], sel_d[1][:])

            # views into pack1
            fT = [pack1[:, OFF_FT + c * 128 : OFF_FT + (c + 1) * 128] for c in range(4)]
            We = [pack1[:, OFF_WE + c * 512 : OFF_WE + (c + 1) * 512] for c in range(4)]
            gT = [pack1[:, OFF_GT + c * 64 : OFF_GT + (c + 1) * 64] for c in range(4)]
            Wd = [pack1[:, OFF_WD + c * 512 : OFF_WD + (c + 1) * 512] for c in range(4)]
            # wj chunk mc, v-half vh  ->  tile mc//2, cols (mc%2)*1024 + vh*512
            wj_t = lambda mc, vh: wjt[mc // 2][:, (mc % 2) * 1024 + vh * 512 : (mc % 2) * 1024 + vh * 512 + 512]
            wj_b = lambda mc, vh: wjb[mc // 2][:, (mc % 2) * 1024 + vh * 512 : (mc % 2) * 1024 + vh * 512 + 512]

            # ---- prologue ----
            with tc.tile_pool(name="pp", bufs=4, space="PSUM") as pp:
                # PE warm-up (see module doc); results never read
                scratch = wp.tile([128, 640], BF16, tag="scratch")
                nc.vector.memset(scratch[:], 1.0)
                wps = pp.tile([128, 512], FP32, tag="warm")
                for _ in range(20):
                    nc.tensor.matmul(
                        wps[:], scratch[:, 0:128], scratch[:, 128:640],
                        start=True, stop=True,
                    )

                # g-path first: C's dependency chain (tgT -> C -> swap) is
                # longer than A's, so let it start as soon as pack1 lands
                for mc in range(4):
                    ms = slice(mc * 128, (mc + 1) * 128)
                    ps = pp.tile([128, U], FP32, tag="pps")
                    for dc in range(4):
                        nc.tensor.matmul(
                            ps[:], Wd[dc][:, ms], gT[dc],
                            start=(dc == 0), stop=(dc == 3),
                        )
                    nc.scalar.activation(
                        tgT[mc][:], ps[:], TANH, bias=bpack[:, 4 + mc : 5 + mc]
                    )
                for mc in range(4):
                    ms = slice(mc * 128, (mc + 1) * 128)
                    ps = pp.tile([128, TC], FP32, tag="pps")
                    for dc in range(4):
                        nc.tensor.matmul(
                            ps[:], We[dc][:, ms], fT[dc],
                            start=(dc == 0), stop=(dc == 3),
                        )
                    nc.scalar.activation(
                        tfT[mc][:], ps[:], TANH, bias=bpack[:, mc : mc + 1]
                    )

                # C once into psum rows 64:128, then swap-replicate
                for vh in range(2):
                    vs = slice(vh * 512, (vh + 1) * 512)
                    ps = pp.tile([128, 512], FP32, tag="pps")
                    for mc in range(4):
                        nc.tensor.matmul(
                            ps[64:128, :], tgT[mc][:], wj_b(mc, vh),
                            start=(mc == 0), stop=False,
                        )
                    nc.tensor.matmul(
                        ps[64:128, :], bjp[:, V : V + 64], bjp[:, vs],
                        start=False, stop=True,
                    )
                    nc.scalar.copy(ACp0[64:128, vs], ps[64:128, :])
                    ps2 = pp.tile([128, 512], FP32, tag="pps")
                    nc.tensor.matmul(
                        ps2[0:64, :],
                        pack1[64:128, OFF_SW : OFF_SW + 64],
                        ACp0[64:128, vs],
                        start=True, stop=True,
                    )
                    nc.vector.tensor_copy(ACp1[0:64, vs], ps2[0:64, :])

                # A = tfT.T @ Wj_top -> ACp0[0:64], ACp1[64:128]
                for vh in range(2):
                    vs = slice(vh * 512, (vh + 1) * 512)
                    ps = pp.tile([128, 512], FP32, tag="pps")
                    for mc in range(4):
                        nc.tensor.matmul(
                            ps[:], tfT[mc][:], wj_t(mc, vh),
                            start=(mc == 0), stop=(mc == 3),
                        )
                    nc.scalar.copy(ACp0[0:64, vs], ps[0:64, :])
                    nc.vector.tensor_copy(ACp1[64:128, vs], ps[64:128, :])

            # ---- main loop: 16 chunks of [512 rows, 1024] bf16 = 1 MB ----
            # chunk j, psum tile a: psO_a[p,:] = out row 512j + 4p + a
            #   -> t = 8j + p//16, u = 4*(p%16) + a, h = j//8
            with (
                tc.tile_pool(name="po", bufs=4, space="PSUM") as po,
                tc.tile_pool(name="ob", bufs=4) as ob,
            ):
                for j in range(16):
                    h, jj = j // 8, j % 8
                    acp = (ACp0, ACp1)[h]
                    out_sb = ob.tile([128, 4 * V], BF16, tag="out")
                    for a in range(4):
                        psO = po.tile([128, V], FP32, tag="psO")
                        c0 = 128 * (4 * jj + a)
                        for vh in range(2):
                            nc.tensor.matmul(
                                psO[:, vh * 512 : (vh + 1) * 512],
                                sel[h][:, c0 : c0 + 128],
                                acp[:, vh * 512 : (vh + 1) * 512],
                                start=True, stop=True,
                            )
                        dst = out_sb[:, a * V : (a + 1) * V]
                        if a % 2 == 0:
                            nc.scalar.copy(dst, psO[:])
                        else:
                            nc.vector.tensor_copy(dst, psO[:])
                        if j in (0, 15) and a == 1:
                            # first/last chunk ship in column halves: the
                            # first write starts two moves earlier and the
                            # final drain tail shrinks by ~1.3 us; the
                            # per-partition DRAM runs are still 4 KB
                            nc.sync.dma_start(
                                out_d[128 * j : 128 * (j + 1), 0 : 2 * V],
                                out_sb[:, 0 : 2 * V],
                            )
                    if j in (0, 15):
                        nc.sync.dma_start(
                            out_d[128 * j : 128 * (j + 1), 2 * V : 4 * V],
                            out_sb[:, 2 * V : 4 * V],
                        )
                    else:
                        nc.sync.dma_start(
                            out_d[128 * j : 128 * (j + 1), :], out_sb[:]
                        )

    nc.compile()
    return nc


def _chunkcat(M):
    """[N*128, C] -> [128, N*C]: stack 128-row chunks side by side."""
    n = M.shape[0] // 128
    return np.ascontiguousarray(
        M.reshape(n, 128, M.shape[1]).transpose(1, 0, 2).reshape(128, -1)
    )


def _build_selectors():
    """Row-permuted pair selectors (see main-loop comment)."""
    sel0 = np.zeros((128, 4096), np.float32)
    sel1 = np.zeros((128, 4096), np.float32)
    p = np.arange(128)
    for jj in range(8):
        for a in range(4):
            col = 128 * (4 * jj + a) + p
            tih = 8 * jj + p // 16
            u = 4 * (p % 16) + a
            sel0[tih, col] = 1.0
            sel0[64 + u, col] = 1.0
            sel1[u, col] = 1.0
            sel1[64 + tih, col] = 1.0
    return sel0.astype(NPFP8), sel1.astype(NPFP8)


def kernel(f, g, We, be, Wd, bd, Wj, bj):
    if "nc" not in _cache:
        _cache["nc"] = _build_nc()
    nc = _cache["nc"]

    b16 = lambda x: np.asarray(x, dtype=np.float32).astype(NPBF16)
    f = np.asarray(f, dtype=np.float32)
    g = np.asarray(g, dtype=np.float32)
    Wj = np.asarray(Wj, dtype=np.float32)

    sel0, sel1 = _build_selectors()
    wjt = _chunkcat(b16(Wj[:D]))
    wjb = _chunkcat(b16(Wj[D:]))
    bjp = np.zeros((1, V + 128), np.float32)
    bjp[0, :V] = np.asarray(bj, dtype=np.float32)
    bjp[0, V:] = 1.0
    bpack = np.zeros((128, 8), np.float32)
    for c in range(4):
        bpack[:, c] = np.asarray(be, dtype=np.float32)[c * 128 : (c + 1) * 128]
        bpack[:, 4 + c] = np.asarray(bd, dtype=np.float32)[c * 128 : (c + 1) * 128]
    We_p = _chunkcat(b16(We))
    Wd_p = _chunkcat(b16(Wd))
    swid = np.zeros((128, 64), np.float32)
    swid[64 + np.arange(64), np.arange(64)] = 1.0

    shared = {
        "wjt0": np.ascontiguousarray(wjt[:, :2048]),
        "wjt1": np.ascontiguousarray(wjt[:, 2048:]),
        "wjb0": np.ascontiguousarray(wjb[:, :2048]),
        "wjb1": np.ascontiguousarray(wjb[:, 2048:]),
        "sel0": sel0, "sel1": sel1,
        "bpack": bpack, "bjp": b16(bjp),
    }
    in_maps = []
    for c in range(NCORES):
        b, th = c // 2, c % 2
        fTp = _chunkcat(b16(f[b, th * TC : (th + 1) * TC, :].T))
        gTp = _chunkcat(b16(g[b].T))
        pack1 = np.concatenate([fTp, We_p, gTp, Wd_p, b16(swid)], axis=1)
        in_maps.append({"pack1": np.ascontiguousarray(pack1), **shared})
    res = run_bass_kernel_spmd(nc, in_maps, list(range(NCORES)))
    kernel._last_results = res

    out = np.empty((B, T, U, V), np.float32)
    for c in range(NCORES):
        b, th = c // 2, c % 2
        out[b, th * TC : (th + 1) * TC] = (
            res.results[c]["out"].astype(np.float32).reshape(TC, U, V)
        )
    return out



# revision 21
# speedup vs baseline: 1.0795x; 1.0795x over previous
"""RNN-T JointNetwork kernel for 8 Trainium2 NeuronCores.

Math: out[b,t,u,:] = tanh(concat(fe[b,t], gd[b,u])) @ Wj + bj
with fe = f@We+be, gd = g@Wd+bd.

Since tanh acts elementwise and the concat feeds a single GEMM, the joint
GEMM factorizes exactly:
    out[b,t,u,:] = A[b,t,:] + C[b,u,:]
    A = tanh(f@We+be) @ Wj[:Dm]          (per-(b,t) row)
    C = tanh(g@Wd+bd) @ Wj[Dm:] + bj     (per-(b,u) row)
This collapses the 137-GFLOP joint GEMM into two tiny GEMMs plus a
broadcast-add, leaving the kernel bound by the output write.

Sharding: 8 cores, core c owns (b = c//2, t-half = c%2) -> a [128,64,V]
output chunk per core.

Trace-driven design (profiled on trn2):
  - HBM reads cap ~290 GB/s (latency-bound; writes sustain ~440) and
    DMAs on one queue complete near-FIFO, so inputs stream in priority
    order: pack_g (g-path gates the longest chain) -> pack_f ->
    wjb_v0 -> wjt_v0 -> sel -> wjb_v1 -> wjt_v1.
  - Everything is pipelined by v-half: the v0 weights land first, the
    v0 half of ACp is built, and a wave of 8 superchunks streams v0
    output while the v1 weights arrive and the v1 prologue GEMMs slot
    between wave superchunks; then the v1 wave.
  - Both ACp tiles use the SAME layout [A-half ; C] so ONE selector
    serves all 16 superchunks.  The A halves land in partitions 0:64
    of separate psum tiles via column-sliced stationary operands; C is
    computed once at partitions 64:128 and copied into both tiles.
  - Superchunk output: partition p owns out rows 8p..8p+8 of a 1024-row
    block = one 8 KB contiguous DRAM run.  The DMA must see a flat 2D
    AP: a [128,8,512] 3-dim view of the same bytes measured ~342 GB/s
    vs ~395-440 for the 2D slice.  Output DRAM is vh-major
    [2*8192, 512]; host reassembles the v halves.
  - psO->SBUF copies only run on ACT (1.11us/[128,1024]) and DVE
    (1.22us) - GpSimd cannot read PSUM on trn2.  Engine is FIXED per
    psum ring slot, and the v1-prologue psum allocations are padded to
    a full ring rotation: an unpadded injection rotates the ring phase
    so every superchunk's first matmul waits on the PREVIOUS
    superchunk's last copy (measured 3.0us/superchunk vs ~2.5).
  - The PE HAM clock gate idles back to ~1.1 GHz after ~1 us of
    inactivity and needs several us of sustained matmuls to re-ramp:
    dummy matmuls bridge the input-wait gaps in the prologue so the
    real GEMMs run near 2.4 GHz (measured 0.63us -> 0.38us / 512 cols).
  - out is bf16: tolerance 2e-2 dwarfs bf16's ~5e-3; host upcasts.
"""

import sys

sys.path.insert(0, "/opt/trn_rl_repo")

import numpy as np

import concourse.bacc as bacc
import concourse.mybir as mybir
import concourse.tile as tile
from concourse.bass_utils import run_bass_kernel_spmd

B, T, U = 4, 256, 64
D = 512  # DE = DD = DM
V = 1024
TC = 128  # t rows per core
NCORES = 8
FP32 = mybir.dt.float32
BF16 = mybir.dt.bfloat16
NPBF16 = mybir.dt.np(mybir.dt.bfloat16)
FP8 = mybir.dt.float8e4
NPFP8 = mybir.dt.np(mybir.dt.float8e4)
TANH = mybir.ActivationFunctionType.Tanh

# pack_g: gT | Wd | biases ; pack_f: fT | We
OFF_GT, OFF_WD, OFF_B = 0, 256, 2304
PACKG_COLS = 2312
OFF_FT, OFF_WE = 0, 512
PACKF_COLS = 2560

_cache = {}


def _build_nc():
    nc = bacc.Bacc("TRN2", target_bir_lowering=False)

    packg_d = nc.dram_tensor("packg", [128, PACKG_COLS], BF16, kind="ExternalInput")
    packf_d = nc.dram_tensor("packf", [128, PACKF_COLS], BF16, kind="ExternalInput")
    brow_d = nc.dram_tensor("brow", [1, V + 64], BF16, kind="ExternalInput")
    wjbv_d = [nc.dram_tensor(f"wjb{v}", [128, 2048], BF16, kind="ExternalInput") for v in range(2)]
    wjtv_d = [nc.dram_tensor(f"wjt{v}", [128, 2048], BF16, kind="ExternalInput") for v in range(2)]
    sel_d = nc.dram_tensor("sel", [128, 4096], FP8, kind="ExternalInput")
    # vh-major: flat row vh*8192 + F holds v-cols vh*512..+512 of out
    # flat row F
    out_d = nc.dram_tensor("out", [2 * 8192, 512], BF16, kind="ExternalOutput")

    with tile.TileContext(nc) as tc:
        with tc.tile_pool(name="wts", bufs=1) as wp:
            packg = wp.tile([128, PACKG_COLS], BF16, tag="packg")
            packf = wp.tile([128, PACKF_COLS], BF16, tag="packf")
            brow = wp.tile([1, V + 64], BF16, tag="brow")
            wjbv = [wp.tile([128, 2048], BF16, tag=f"wjb{v}", name=f"wjb{v}") for v in range(2)]
            wjtv = [wp.tile([128, 2048], BF16, tag=f"wjt{v}", name=f"wjt{v}") for v in range(2)]
            sel = wp.tile([128, 4096], FP8, tag="sel")
            tfT = [wp.tile([128, TC], BF16, tag=f"tfT{c}", name=f"tfT{c}") for c in range(4)]
            tgT = [wp.tile([128, U], BF16, tag=f"tgT{c}", name=f"tgT{c}") for c in range(4)]
            ACp0 = wp.tile([128, V], BF16, tag="ACp0")
            ACp1 = wp.tile([128, V], BF16, tag="ACp1")

            # input stream: arrival priority = issue order (near-FIFO)
            nc.sync.dma_start(packg[:], packg_d[:])
            nc.sync.dma_start(brow[:], brow_d[:])
            nc.sync.dma_start(packf[:], packf_d[:])
            nc.sync.dma_start(wjbv[0][:], wjbv_d[0][:])
            nc.sync.dma_start(wjtv[0][:], wjtv_d[0][:])
            nc.sync.dma_start(sel[:], sel_d[:])
            nc.sync.dma_start(wjbv[1][:], wjbv_d[1][:])
            nc.sync.dma_start(wjtv[1][:], wjtv_d[1][:])

            fT = [packf[:, OFF_FT + c * 128 : OFF_FT + (c + 1) * 128] for c in range(4)]
            We = [packf[:, OFF_WE + c * 512 : OFF_WE + (c + 1) * 512] for c in range(4)]
            gT = [packg[:, OFF_GT + c * 64 : OFF_GT + (c + 1) * 64] for c in range(4)]
            Wd = [packg[:, OFF_WD + c * 512 : OFF_WD + (c + 1) * 512] for c in range(4)]
            be = lambda mc: packg[:, OFF_B + mc : OFF_B + mc + 1]
            bd = lambda mc: packg[:, OFF_B + 4 + mc : OFF_B + 5 + mc]
            wj_b = lambda mc, vh: wjbv[vh][:, 512 * mc : 512 * mc + 512]
            wj_t = lambda mc, vh: wjtv[vh][:, 512 * mc : 512 * mc + 512]
            ones64 = brow[:, V : V + 64]
            bj = lambda vh: brow[:, vh * 512 : (vh + 1) * 512]

            def c_gemm(ps, vh):
                """C v-half into psum partitions 64:128 (+bj)."""
                for mc in range(4):
                    nc.tensor.matmul(
                        ps[64:128, :], tgT[mc][:], wj_b(mc, vh),
                        start=(mc == 0), stop=False,
                    )
                nc.tensor.matmul(
                    ps[64:128, :], ones64, bj(vh), start=False, stop=True
                )

            def a_gemm(ps, h, vh):
                """A t-half h, v-half vh into psum partitions 0:64."""
                hs = slice(h * 64, (h + 1) * 64)
                for mc in range(4):
                    nc.tensor.matmul(
                        ps[0:64, :], tfT[mc][:, hs], wj_t(mc, vh),
                        start=(mc == 0), stop=(mc == 3),
                    )

            def c_copies(ps, vh):
                vs = slice(vh * 512, (vh + 1) * 512)
                nc.scalar.copy(ACp0[64:128, vs], ps[64:128, :])
                nc.vector.tensor_copy(ACp1[64:128, vs], ps[64:128, :])

            def a_copy(ps, h, vh):
                vs = slice(vh * 512, (vh + 1) * 512)
                acp = (ACp0, ACp1)[h]
                if h == 0:
                    nc.scalar.copy(acp[0:64, vs], ps[0:64, :])
                else:
                    nc.vector.tensor_copy(acp[0:64, vs], ps[0:64, :])

            # ---- v0 prologue ----
            with tc.tile_pool(name="pp", bufs=1, space="PSUM") as pp:
                # PE warm-up; results never read.  Interleaved with the
                # real GEMM groups so the HAM clock never idles down
                # during input-wait gaps (see module doc).
                scratch = wp.tile([128, 640], BF16, tag="scratch")
                nc.vector.memset(scratch[:], 1.0)
                wps = pp.tile([128, 512], FP32, tag="pps", bufs=4)

                def warm(n):
                    for _ in range(n):
                        nc.tensor.matmul(
                            wps[:], scratch[:, 0:128], scratch[:, 128:640],
                            start=True, stop=True,
                        )

                warm(4)
                # g-path first: C's dependency chain is longest
                for mc in range(4):
                    ms = slice(mc * 128, (mc + 1) * 128)
                    ps = pp.tile([128, U], FP32, tag="pps", bufs=4)
                    for dc in range(4):
                        nc.tensor.matmul(
                            ps[:], Wd[dc][:, ms], gT[dc],
                            start=(dc == 0), stop=(dc == 3),
                        )
                    nc.scalar.activation(tgT[mc][:], ps[:], TANH, bias=bd(mc))
                warm(3)  # bridge the wait for packf
                for mc in range(4):
                    ms = slice(mc * 128, (mc + 1) * 128)
                    ps = pp.tile([128, TC], FP32, tag="pps", bufs=4)
                    for dc in range(4):
                        nc.tensor.matmul(
                            ps[:], We[dc][:, ms], fT[dc],
                            start=(dc == 0), stop=(dc == 3),
                        )
                    nc.scalar.activation(tfT[mc][:], ps[:], TANH, bias=be(mc))
                warm(2)  # bridge the wait for wjb_v0

                psC0 = pp.tile([128, 512], FP32, tag="pj", bufs=3)
                c_gemm(psC0, 0)
                c_copies(psC0, 0)
                warm(1)  # bridge the wait for wjt_v0
                psA00 = pp.tile([128, 512], FP32, tag="pj", bufs=3)
                a_gemm(psA00, 0, 0)
                a_copy(psA00, 0, 0)
                # A-h1-v0 is NOT built here: superchunk 0 needs only
                # ACp0-v0, so it is injected after S0 (needed at J=4)

            # ---- main loop: 2 waves x 8 superchunks of [1024 rows, 512]
            # superchunk (vh, J), psum slot q: psO_q[p,:] = v-cols
            # vh*512..+512 of out flat row 1024J + 8p + q
            #   -> t = 16J + p//8, u = 8*(p%8) + q
            with (
                tc.tile_pool(name="po", bufs=1, space="PSUM") as po,
                tc.tile_pool(name="ob", bufs=5) as ob,
            ):
                def pot():
                    return po.tile([128, 1024], FP32, tag="psO", bufs=4, name="psO")

                for vh in range(2):
                    for J in range(8):
                        first = vh == 0 and J == 0
                        last = vh == 1 and J == 7
                        JJ = J % 4
                        acp = (ACp0, ACp1)[J // 4]
                        vs = slice(vh * 512, (vh + 1) * 512)
                        r0 = 8192 * vh + 1024 * J
                        out_sb = ob.tile([128, 4096], BF16, tag="out")
                        for k in range(4):
                            psO = pot()
                            for i in range(2):
                                q = 2 * k + i
                                c0 = 128 * (8 * JJ + q)
                                nc.tensor.matmul(
                                    psO[:, i * 512 : (i + 1) * 512],
                                    sel[:, c0 : c0 + 128],
                                    acp[:, vs],
                                    start=True, stop=True,
                                )
                            dst = out_sb[:, k * 1024 : (k + 1) * 1024]
                            # engine FIXED per slot k: ACT owns the two
                            # EARLY slots so the next superchunk's first
                            # matmul (WAR on slot 0) unblocks off ACT's
                            # first copy; DVE owns the late slots and
                            # gates only the DMA issue
                            if k < 2:
                                nc.scalar.copy(dst, psO[:])
                            else:
                                nc.vector.tensor_copy(dst, psO[:])
                            if (first or last) and k == 1:
                                # first/last superchunk ship in q-halves
                                # (earlier first write / shorter drain);
                                # 4 KB strided runs via a rearranged AP
                                ov = out_d[r0 : r0 + 1024, :].rearrange(
                                    "(p q) v -> p (q v)", q=8
                                )
                                nc.sync.dma_start(
                                    ov[:, 0:2048], out_sb[:, 0:2048]
                                )
                        if first or last:
                            ov = out_d[r0 : r0 + 1024, :].rearrange(
                                "(p q) v -> p (q v)", q=8
                            )
                            nc.sync.dma_start(
                                ov[:, 2048:4096], out_sb[:, 2048:4096]
                            )
                        else:
                            nc.sync.dma_start(
                                out_d[r0 : r0 + 1024, :], out_sb[:]
                            )

                        # v1 prologue GEMMs slot between early v0-wave
                        # superchunks once wj_v1 has landed; each
                        # injection pads to a FULL psO ring rotation so
                        # the slot->engine phase is preserved
                        if vh == 0 and J in (0, 1, 3, 5):
                            aux = pot()[:, 0:512]
                            if J == 0:
                                a_gemm(aux, 1, 0)
                                a_copy(aux, 1, 0)
                            elif J == 1:
                                c_gemm(aux, 1)
                                c_copies(aux, 1)
                            elif J == 3:
                                a_gemm(aux, 0, 1)
                                a_copy(aux, 0, 1)
                            else:
                                a_gemm(aux, 1, 1)
                                a_copy(aux, 1, 1)
                            for _ in range(3):
                                pot()  # phase padding, never touched

    nc.compile()
    return nc


def _chunkcat(M):
    """[N*128, C] -> [128, N*C]: stack 128-row chunks side by side."""
    n = M.shape[0] // 128
    return np.ascontiguousarray(
        M.reshape(n, 128, M.shape[1]).transpose(1, 0, 2).reshape(128, -1)
    )


def _build_selector():
    """Row-permuted pair selector (see main-loop comment)."""
    sel = np.zeros((128, 4096), np.float32)
    p = np.arange(128)
    for JJ in range(4):
        for q in range(8):
            col = 128 * (8 * JJ + q) + p
            sel[16 * JJ + p // 8, col] = 1.0
            sel[64 + 8 * (p % 8) + q, col] = 1.0
    return sel.astype(NPFP8)


def kernel(f, g, We, be, Wd, bd, Wj, bj):
    if "nc" not in _cache:
        _cache["nc"] = _build_nc()
    nc = _cache["nc"]

    b16 = lambda x: np.asarray(x, dtype=np.float32).astype(NPBF16)
    f = np.asarray(f, dtype=np.float32)
    g = np.asarray(g, dtype=np.float32)
    Wj = np.asarray(Wj, dtype=np.float32)

    sel = _build_selector()
    wjt = _chunkcat(b16(Wj[:D])).reshape(128, 4, 2, 512)  # [p, mc, vh, v]
    wjb = _chunkcat(b16(Wj[D:])).reshape(128, 4, 2, 512)
    brow = np.zeros((1, V + 64), np.float32)
    brow[0, :V] = np.asarray(bj, dtype=np.float32)
    brow[0, V:] = 1.0
    bias8 = np.zeros((128, 8), np.float32)
    for c in range(4):
        bias8[:, c] = np.asarray(be, dtype=np.float32)[c * 128 : (c + 1) * 128]
        bias8[:, 4 + c] = np.asarray(bd, dtype=np.float32)[c * 128 : (c + 1) * 128]
    We_p = _chunkcat(b16(We))
    Wd_p = _chunkcat(b16(Wd))

    shared = {"sel": sel, "brow": b16(brow)}
    for v in range(2):
        shared[f"wjt{v}"] = np.ascontiguousarray(wjt[:, :, v, :].reshape(128, 2048))
        shared[f"wjb{v}"] = np.ascontiguousarray(wjb[:, :, v, :].reshape(128, 2048))
    in_maps = []
    for c in range(NCORES):
        b, th = c // 2, c % 2
        fTp = _chunkcat(b16(f[b, th * TC : (th + 1) * TC, :].T))
        gTp = _chunkcat(b16(g[b].T))
        packg = np.concatenate([gTp, Wd_p, b16(bias8)], axis=1)
        packf = np.concatenate([fTp, We_p], axis=1)
        in_maps.append({
            "packg": np.ascontiguousarray(packg),
            "packf": np.ascontiguousarray(packf),
            **shared,
        })
    res = run_bass_kernel_spmd(nc, in_maps, list(range(NCORES)))
    kernel._last_results = res

    out = np.empty((B, T, U, V), np.float32)
    for c in range(NCORES):
        b, th = c // 2, c % 2
        raw = res.results[c]["out"].astype(np.float32).reshape(2 * 8192, 512)
        half = np.empty((8192, V), np.float32)
        half[:, 0:512] = raw[0:8192]
        half[:, 512:1024] = raw[8192:16384]
        out[b, th * TC : (th + 1) * TC] = half.reshape(TC, U, V)
    return out


# revision 25
# speedup vs baseline: 1.0831x; 1.0033x over previous
"""RNN-T JointNetwork kernel for 8 Trainium2 NeuronCores.

Math: out[b,t,u,:] = tanh(concat(fe[b,t], gd[b,u])) @ Wj + bj
with fe = f@We+be, gd = g@Wd+bd.

Since tanh acts elementwise and the concat feeds a single GEMM, the joint
GEMM factorizes exactly:
    out[b,t,u,:] = A[b,t,:] + C[b,u,:]
    A = tanh(f@We+be) @ Wj[:Dm]          (per-(b,t) row)
    C = tanh(g@Wd+bd) @ Wj[Dm:] + bj     (per-(b,u) row)
This collapses the 137-GFLOP joint GEMM into two tiny GEMMs plus a
broadcast-add, leaving the kernel bound by the output write.

Sharding: 8 cores, core c owns (b = c//2, t-half = c%2) -> a [128,64,V]
output chunk per core.

Trace-driven design (profiled on trn2):
  - HBM reads cap ~290 GB/s (latency-bound; writes sustain ~440) and
    DMAs on one queue complete near-FIFO, so inputs stream in priority
    order: pack_g (g-path gates the longest chain) -> pack_f ->
    wjb_v0 -> wjt_v0 -> sel -> wjb_v1 -> wjt_v1.
  - Everything is pipelined by v-half: the v0 weights land first, the
    v0 half of ACp is built, and a wave of 8 superchunks streams v0
    output while the v1 weights arrive and the v1 prologue GEMMs slot
    between wave superchunks; then the v1 wave.
  - Both ACp tiles use the SAME layout [A-half ; C] so ONE selector
    serves all 16 superchunks.  The A halves land in partitions 0:64
    of separate psum tiles via column-sliced stationary operands; C is
    computed once at partitions 64:128 and copied into both tiles.
  - Superchunk output: partition p owns out rows 8p..8p+8 of a 1024-row
    block = one 8 KB contiguous DRAM run.  The DMA must see a flat 2D
    AP: a [128,8,512] 3-dim view of the same bytes measured ~342 GB/s
    vs ~395-440 for the 2D slice.  Output DRAM is vh-major
    [2*8192, 512]; host reassembles the v halves.
  - psO->SBUF copies only run on ACT (1.11us/[128,1024]) and DVE
    (1.22us) - GpSimd cannot read PSUM on trn2.  Engine is FIXED per
    psum ring slot, and the v1-prologue psum allocations are padded to
    a full ring rotation: an unpadded injection rotates the ring phase
    so every superchunk's first matmul waits on the PREVIOUS
    superchunk's last copy (measured 3.0us/superchunk vs ~2.5).
  - The PE HAM clock gate idles back to ~1.1 GHz after ~1 us of
    inactivity and needs several us of sustained matmuls to re-ramp:
    dummy matmuls bridge the input-wait gaps in the prologue so the
    real GEMMs run near 2.4 GHz (measured 0.63us -> 0.38us / 512 cols).
  - out is bf16: tolerance 2e-2 dwarfs bf16's ~5e-3; host upcasts.
"""

import sys

sys.path.insert(0, "/opt/trn_rl_repo")

import numpy as np

import concourse.bacc as bacc
import concourse.mybir as mybir
import concourse.tile as tile
from concourse.bass_utils import run_bass_kernel_spmd

B, T, U = 4, 256, 64
D = 512  # DE = DD = DM
V = 1024
TC = 128  # t rows per core
NCORES = 8
FP32 = mybir.dt.float32
BF16 = mybir.dt.bfloat16
NPBF16 = mybir.dt.np(mybir.dt.bfloat16)
FP8 = mybir.dt.float8e4
NPFP8 = mybir.dt.np(mybir.dt.float8e4)
TANH = mybir.ActivationFunctionType.Tanh

# pack_g: gT | Wd | biases ; pack_f: fT | We
OFF_GT, OFF_WD, OFF_B = 0, 256, 2304
PACKG_COLS = 2312
OFF_FT, OFF_WE = 0, 512
PACKF_COLS = 2560

_cache = {}


def _build_nc():
    nc = bacc.Bacc("TRN2", target_bir_lowering=False)

    packg_d = nc.dram_tensor("packg", [128, PACKG_COLS], BF16, kind="ExternalInput")
    packf_d = nc.dram_tensor("packf", [128, PACKF_COLS], BF16, kind="ExternalInput")
    brow_d = nc.dram_tensor("brow", [1, V + 64], BF16, kind="ExternalInput")
    wjbv_d = [nc.dram_tensor(f"wjb{v}", [128, 2048], BF16, kind="ExternalInput") for v in range(2)]
    wjtv_d = [nc.dram_tensor(f"wjt{v}", [128, 2048], BF16, kind="ExternalInput") for v in range(2)]
    sel_d = nc.dram_tensor("sel", [128, 4096], FP8, kind="ExternalInput")
    # vh-major: flat row vh*8192 + F holds v-cols vh*512..+512 of out
    # flat row F; declared [2048, 4096] (row = 8 flat rows = one
    # partition's 8 KB run) so the AP normalizer emits single 8 KB
    # descriptors - a [1024, 512] view of the same bytes generated
    # per-row 1 KB descriptors and only ~330-370 GB/s
    out_d = nc.dram_tensor("out", [2048, 4096], BF16, kind="ExternalOutput")

    with tile.TileContext(nc) as tc:
        with tc.tile_pool(name="wts", bufs=1) as wp:
            packg = wp.tile([128, PACKG_COLS], BF16, tag="packg")
            packf = wp.tile([128, PACKF_COLS], BF16, tag="packf")
            brow = wp.tile([1, V + 64], BF16, tag="brow")
            wjbv = [wp.tile([128, 2048], BF16, tag=f"wjb{v}", name=f"wjb{v}") for v in range(2)]
            wjtv = [wp.tile([128, 2048], BF16, tag=f"wjt{v}", name=f"wjt{v}") for v in range(2)]
            sel = wp.tile([128, 4096], FP8, tag="sel")
            tfT = [wp.tile([128, TC], BF16, tag=f"tfT{c}", name=f"tfT{c}") for c in range(4)]
            tgT = [wp.tile([128, U], BF16, tag=f"tgT{c}", name=f"tgT{c}") for c in range(4)]
            ACp0 = wp.tile([128, V], BF16, tag="ACp0")
            ACp1 = wp.tile([128, V], BF16, tag="ACp1")

            # input stream: arrival priority = issue order (near-FIFO)
            nc.sync.dma_start(packg[:], packg_d[:])
            nc.sync.dma_start(brow[:], brow_d[:])
            nc.sync.dma_start(packf[:], packf_d[:])
            nc.sync.dma_start(wjbv[0][:], wjbv_d[0][:])
            nc.sync.dma_start(wjtv[0][:], wjtv_d[0][:])
            nc.sync.dma_start(sel[:], sel_d[:])
            nc.sync.dma_start(wjbv[1][:], wjbv_d[1][:])
            nc.sync.dma_start(wjtv[1][:], wjtv_d[1][:])

            fT = [packf[:, OFF_FT + c * 128 : OFF_FT + (c + 1) * 128] for c in range(4)]
            We = [packf[:, OFF_WE + c * 512 : OFF_WE + (c + 1) * 512] for c in range(4)]
            gT = [packg[:, OFF_GT + c * 64 : OFF_GT + (c + 1) * 64] for c in range(4)]
            Wd = [packg[:, OFF_WD + c * 512 : OFF_WD + (c + 1) * 512] for c in range(4)]
            be = lambda mc: packg[:, OFF_B + mc : OFF_B + mc + 1]
            bd = lambda mc: packg[:, OFF_B + 4 + mc : OFF_B + 5 + mc]
            wj_b = lambda mc, vh: wjbv[vh][:, 512 * mc : 512 * mc + 512]
            wj_t = lambda mc, vh: wjtv[vh][:, 512 * mc : 512 * mc + 512]
            ones64 = brow[:, V : V + 64]
            bj = lambda vh: brow[:, vh * 512 : (vh + 1) * 512]

            def c_gemm(ps, vh):
                """C v-half into psum partitions 64:128 (+bj)."""
                for mc in range(4):
                    nc.tensor.matmul(
                        ps[64:128, :], tgT[mc][:], wj_b(mc, vh),
                        start=(mc == 0), stop=False,
                    )
                nc.tensor.matmul(
                    ps[64:128, :], ones64, bj(vh), start=False, stop=True
                )

            def a_gemm(ps, h, vh):
                """A t-half h, v-half vh into psum partitions 0:64."""
                hs = slice(h * 64, (h + 1) * 64)
                for mc in range(4):
                    nc.tensor.matmul(
                        ps[0:64, :], tfT[mc][:, hs], wj_t(mc, vh),
                        start=(mc == 0), stop=(mc == 3),
                    )

            def c_copies(ps, vh):
                vs = slice(vh * 512, (vh + 1) * 512)
                nc.scalar.copy(ACp0[64:128, vs], ps[64:128, :])
                nc.vector.tensor_copy(ACp1[64:128, vs], ps[64:128, :])

            def a_copy(ps, h, vh):
                vs = slice(vh * 512, (vh + 1) * 512)
                acp = (ACp0, ACp1)[h]
                if h == 0:
                    nc.scalar.copy(acp[0:64, vs], ps[0:64, :])
                else:
                    nc.vector.tensor_copy(acp[0:64, vs], ps[0:64, :])

            # ---- v0 prologue ----
            with tc.tile_pool(name="pp", bufs=1, space="PSUM") as pp:
                # PE warm-up; results never read.  Interleaved with the
                # real GEMM groups so the HAM clock never idles down
                # during input-wait gaps (see module doc).
                scratch = wp.tile([128, 640], BF16, tag="scratch")
                nc.vector.memset(scratch[:], 1.0)
                wps = pp.tile([128, 512], FP32, tag="pps", bufs=4)

                def warm(n):
                    for _ in range(n):
                        nc.tensor.matmul(
                            wps[:], scratch[:, 0:128], scratch[:, 128:640],
                            start=True, stop=True,
                        )

                warm(4)
                # g-path first: C's dependency chain is longest
                for mc in range(4):
                    ms = slice(mc * 128, (mc + 1) * 128)
                    ps = pp.tile([128, U], FP32, tag="pps", bufs=4)
                    for dc in range(4):
                        nc.tensor.matmul(
                            ps[:], Wd[dc][:, ms], gT[dc],
                            start=(dc == 0), stop=(dc == 3),
                        )
                    nc.scalar.activation(tgT[mc][:], ps[:], TANH, bias=bd(mc))
                warm(3)  # bridge the wait for packf
                for mc in range(4):
                    ms = slice(mc * 128, (mc + 1) * 128)
                    ps = pp.tile([128, TC], FP32, tag="pps", bufs=4)
                    for dc in range(4):
                        nc.tensor.matmul(
                            ps[:], We[dc][:, ms], fT[dc],
                            start=(dc == 0), stop=(dc == 3),
                        )
                    nc.scalar.activation(tfT[mc][:], ps[:], TANH, bias=be(mc))
                warm(2)  # bridge the wait for wjb_v0

                psC0 = pp.tile([128, 512], FP32, tag="pj", bufs=3)
                c_gemm(psC0, 0)
                c_copies(psC0, 0)
                warm(1)  # bridge the wait for wjt_v0
                psA00 = pp.tile([128, 512], FP32, tag="pj", bufs=3)
                a_gemm(psA00, 0, 0)
                a_copy(psA00, 0, 0)
                # A-h1-v0 is NOT built here: superchunk 0 needs only
                # ACp0-v0, so it is injected after S0 (needed at J=4)

            # ---- main loop: 2 waves x 8 superchunks of [1024 rows, 512]
            # superchunk (vh, J), psum slot q: psO_q[p,:] = v-cols
            # vh*512..+512 of out flat row 1024J + 8p + q
            #   -> t = 16J + p//8, u = 8*(p%8) + q
            with (
                tc.tile_pool(name="po", bufs=1, space="PSUM") as po,
                tc.tile_pool(name="ob", bufs=5) as ob,
            ):
                def pot():
                    return po.tile([128, 1024], FP32, tag="psO", bufs=4, name="psO")

                for vh in range(2):
                    for J in range(8):
                        first = vh == 0 and J == 0
                        last = vh == 1 and J == 7
                        JJ = J % 4
                        acp = (ACp0, ACp1)[J // 4]
                        vs = slice(vh * 512, (vh + 1) * 512)
                        r0 = 1024 * vh + 128 * J
                        out_sb = ob.tile([128, 4096], BF16, tag="out")
                        for k in range(4):
                            psO = pot()
                            for i in range(2):
                                q = 2 * k + i
                                c0 = 128 * (8 * JJ + q)
                                nc.tensor.matmul(
                                    psO[:, i * 512 : (i + 1) * 512],
                                    sel[:, c0 : c0 + 128],
                                    acp[:, vs],
                                    start=True, stop=True,
                                )
                            dst = out_sb[:, k * 1024 : (k + 1) * 1024]
                            # engine FIXED per slot k: ACT owns the two
                            # EARLY slots so the next superchunk's first
                            # matmul (WAR on slot 0) unblocks off ACT's
                            # first copy; DVE owns the late slots and
                            # gates only the DMA issue
                            if k < 2:
                                nc.scalar.copy(dst, psO[:])
                            else:
                                nc.vector.tensor_copy(dst, psO[:])
                            if (first or last) and k == 1:
                                # first/last superchunk ship in q-halves
                                # (earlier first write / shorter drain);
                                # plain column slice = 4 KB runs
                                nc.sync.dma_start(
                                    out_d[r0 : r0 + 128, 0:2048],
                                    out_sb[:, 0:2048],
                                )
                        if first or last:
                            nc.sync.dma_start(
                                out_d[r0 : r0 + 128, 2048:4096],
                                out_sb[:, 2048:4096],
                            )
                        else:
                            nc.sync.dma_start(
                                out_d[r0 : r0 + 128, :], out_sb[:]
                            )

                        # v1 prologue GEMMs slot between early v0-wave
                        # superchunks once wj_v1 has landed; each
                        # injection pads to a FULL psO ring rotation so
                        # the slot->engine phase is preserved
                        if vh == 0 and J in (0, 1, 3, 5):
                            aux = pot()[:, 0:512]
                            if J == 0:
                                a_gemm(aux, 1, 0)
                                a_copy(aux, 1, 0)
                            elif J == 1:
                                c_gemm(aux, 1)
                                c_copies(aux, 1)
                            elif J == 3:
                                a_gemm(aux, 0, 1)
                                a_copy(aux, 0, 1)
                            else:
                                a_gemm(aux, 1, 1)
                                a_copy(aux, 1, 1)
                            for _ in range(3):
                                pot()  # phase padding, never touched

    nc.compile()
    return nc


def _chunkcat(M):
    """[N*128, C] -> [128, N*C]: stack 128-row chunks side by side."""
    n = M.shape[0] // 128
    return np.ascontiguousarray(
        M.reshape(n, 128, M.shape[1]).transpose(1, 0, 2).reshape(128, -1)
    )


def _build_selector():
    """Row-permuted pair selector (see main-loop comment)."""
    sel = np.zeros((128, 4096), np.float32)
    p = np.arange(128)
    for JJ in range(4):
        for q in range(8):
            col = 128 * (8 * JJ + q) + p
            sel[16 * JJ + p // 8, col] = 1.0
            sel[64 + 8 * (p % 8) + q, col] = 1.0
    return sel.astype(NPFP8)


def kernel(f, g, We, be, Wd, bd, Wj, bj):
    if "nc" not in _cache:
        _cache["nc"] = _build_nc()
    nc = _cache["nc"]

    b16 = lambda x: np.asarray(x, dtype=np.float32).astype(NPBF16)
    f = np.asarray(f, dtype=np.float32)
    g = np.asarray(g, dtype=np.float32)
    Wj = np.asarray(Wj, dtype=np.float32)

    sel = _build_selector()
    wjt = _chunkcat(b16(Wj[:D])).reshape(128, 4, 2, 512)  # [p, mc, vh, v]
    wjb = _chunkcat(b16(Wj[D:])).reshape(128, 4, 2, 512)
    brow = np.zeros((1, V + 64), np.float32)
    brow[0, :V] = np.asarray(bj, dtype=np.float32)
    brow[0, V:] = 1.0
    bias8 = np.zeros((128, 8), np.float32)
    for c in range(4):
        bias8[:, c] = np.asarray(be, dtype=np.float32)[c * 128 : (c + 1) * 128]
        bias8[:, 4 + c] = np.asarray(bd, dtype=np.float32)[c * 128 : (c + 1) * 128]
    We_p = _chunkcat(b16(We))
    Wd_p = _chunkcat(b16(Wd))

    shared = {"sel": sel, "brow": b16(brow)}
    for v in range(2):
        shared[f"wjt{v}"] = np.ascontiguousarray(wjt[:, :, v, :].reshape(128, 2048))
        shared[f"wjb{v}"] = np.ascontiguousarray(wjb[:, :, v, :].reshape(128, 2048))
    in_maps = []
    for c in range(NCORES):
        b, th = c // 2, c % 2
        fTp = _chunkcat(b16(f[b, th * TC : (th + 1) * TC, :].T))
        gTp = _chunkcat(b16(g[b].T))
        packg = np.concatenate([gTp, Wd_p, b16(bias8)], axis=1)
        packf = np.concatenate([fTp, We_p], axis=1)
        in_maps.append({
            "packg": np.ascontiguousarray(packg),
            "packf": np.ascontiguousarray(packf),
            **shared,
        })
    res = run_bass_kernel_spmd(nc, in_maps, list(range(NCORES)))
    kernel._last_results = res

    out = np.empty((B, T, U, V), np.float32)
    for c in range(NCORES):
        b, th = c // 2, c % 2
        raw = res.results[c]["out"].astype(np.float32).reshape(2 * 8192, 512)  # [2048,4096] -> flat
        half = np.empty((8192, V), np.float32)
        half[:, 0:512] = raw[0:8192]
        half[:, 512:1024] = raw[8192:16384]
        out[b, th * TC : (th + 1) * TC] = half.reshape(TC, U, V)
    return out


# revision 30
# speedup vs baseline: 1.1513x; 1.0630x over previous
"""RNN-T JointNetwork kernel for 8 Trainium2 NeuronCores.

Math: out[b,t,u,:] = tanh(concat(fe[b,t], gd[b,u])) @ Wj + bj
with fe = f@We+be, gd = g@Wd+bd.

Since tanh acts elementwise and the concat feeds a single GEMM, the joint
GEMM factorizes exactly:
    out[b,t,u,:] = A[b,t,:] + C[b,u,:]
    A = tanh(f@We+be) @ Wj[:Dm]          (per-(b,t) row)
    C = tanh(g@Wd+bd) @ Wj[Dm:] + bj     (per-(b,u) row)
This collapses the 137-GFLOP joint GEMM into two tiny GEMMs plus a
broadcast-add, leaving the kernel bound by the output write.

Sharding: 8 cores, core c owns (b = c//2, t-half = c%2) -> a [128,64,V]
output chunk per core.

Trace-driven design (profiled on trn2):
  - HBM reads cap ~290 GB/s (latency-bound; writes sustain ~440) and
    DMAs on one queue complete near-FIFO, so inputs stream in priority
    order: pack_g (g-path gates the longest chain) -> pack_f ->
    wjb_v0 -> wjt_v0 -> sel -> wjb_v1 -> wjt_v1.
  - Everything is pipelined by v-half: the v0 weights land first, the
    v0 half of ACp is built, and a wave of 8 superchunks streams v0
    output while the v1 weights arrive and the v1 prologue GEMMs slot
    between wave superchunks; then the v1 wave.
  - Both ACp tiles use the SAME layout [A-half ; C] so ONE selector
    serves all 16 superchunks.  The A halves land in partitions 0:64
    of separate psum tiles via column-sliced stationary operands; C is
    computed once at partitions 64:128 and copied into both tiles.
  - Superchunk output: partition p owns out rows 8p..8p+8 of a 1024-row
    block = one 8 KB contiguous DRAM run.  The DMA must see a flat 2D
    AP: a [128,8,512] 3-dim view of the same bytes measured ~342 GB/s
    vs ~395-440 for the 2D slice.  Output DRAM is vh-major
    [2*8192, 512]; host reassembles the v halves.
  - psO->SBUF copies only run on ACT (1.11us/[128,1024]) and DVE
    (1.22us) - GpSimd cannot read PSUM on trn2.  Engine is FIXED per
    psum ring slot, and the v1-prologue psum allocations are padded to
    a full ring rotation: an unpadded injection rotates the ring phase
    so every superchunk's first matmul waits on the PREVIOUS
    superchunk's last copy (measured 3.0us/superchunk vs ~2.5).
  - The PE HAM clock gate idles back to ~1.1 GHz after ~1 us of
    inactivity and needs several us of sustained matmuls to re-ramp:
    dummy matmuls bridge the input-wait gaps in the prologue so the
    real GEMMs run near 2.4 GHz (measured 0.63us -> 0.38us / 512 cols).
  - out is bf16: tolerance 2e-2 dwarfs bf16's ~5e-3; host upcasts.
"""

import sys

sys.path.insert(0, "/opt/trn_rl_repo")

import numpy as np

import concourse.bacc as bacc
import concourse.mybir as mybir
import concourse.tile as tile
from concourse.bass_utils import run_bass_kernel_spmd

B, T, U = 4, 256, 64
D = 512  # DE = DD = DM
V = 1024
TC = 128  # t rows per core
NCORES = 8
FP32 = mybir.dt.float32
BF16 = mybir.dt.bfloat16
NPBF16 = mybir.dt.np(mybir.dt.bfloat16)
FP8 = mybir.dt.float8e4
NPFP8 = mybir.dt.np(mybir.dt.float8e4)
TANH = mybir.ActivationFunctionType.Tanh

# pack_g: gT | Wd | biases ; pack_f: fT | We
OFF_GT, OFF_WD, OFF_B = 0, 256, 2304
PACKG_COLS = 2312
OFF_FT, OFF_WE = 0, 512
PACKF_COLS = 2560

_cache = {}


def _build_nc():
    nc = bacc.Bacc("TRN2", target_bir_lowering=False)

    packg_d = nc.dram_tensor("packg", [128, PACKG_COLS], BF16, kind="ExternalInput")
    packf_d = nc.dram_tensor("packf", [128, PACKF_COLS], BF16, kind="ExternalInput")
    brow_d = nc.dram_tensor("brow", [1, V + 64], BF16, kind="ExternalInput")
    wjbv_d = [nc.dram_tensor(f"wjb{v}", [128, 2048], BF16, kind="ExternalInput") for v in range(2)]
    wjtv_d = [nc.dram_tensor(f"wjt{v}", [128, 2048], BF16, kind="ExternalInput") for v in range(2)]
    sel_d = nc.dram_tensor("sel", [128, 4096], FP8, kind="ExternalInput")
    # vh-major, superchunk-PAIRED: row R0+p of a pair region holds
    # [S_J's 8 q-blocks | S_J+1's 8 q-blocks] for partition p = one
    # 16 KB contiguous run (1 MB transfers measured only ~370-390 GB/s;
    # bigger per-partition runs amortize the per-DMA ramp).  The DMA
    # must see a flat 1:1 2D AP - shape-mismatched views measured
    # ~330-342 GB/s.  Host unscrambles the static permutation.
    out_d = nc.dram_tensor("out", [1024, 8192], BF16, kind="ExternalOutput")

    with tile.TileContext(nc) as tc:
        with tc.tile_pool(name="wts", bufs=1) as wp:
            packg = wp.tile([128, PACKG_COLS], BF16, tag="packg")
            packf = wp.tile([128, PACKF_COLS], BF16, tag="packf")
            brow = wp.tile([1, V + 64], BF16, tag="brow")
            wjbv = [wp.tile([128, 2048], BF16, tag=f"wjb{v}", name=f"wjb{v}") for v in range(2)]
            wjtv = [wp.tile([128, 2048], BF16, tag=f"wjt{v}", name=f"wjt{v}") for v in range(2)]
            sel = wp.tile([128, 4096], FP8, tag="sel")
            tfT = [wp.tile([128, TC], BF16, tag=f"tfT{c}", name=f"tfT{c}") for c in range(4)]
            tgT = [wp.tile([128, U], BF16, tag=f"tgT{c}", name=f"tgT{c}") for c in range(4)]
            ACp0 = wp.tile([128, V], BF16, tag="ACp0")
            ACp1 = wp.tile([128, V], BF16, tag="ACp1")

            # input stream: arrival priority = issue order (near-FIFO)
            nc.sync.dma_start(packg[:], packg_d[:])
            nc.sync.dma_start(brow[:], brow_d[:])
            nc.sync.dma_start(packf[:], packf_d[:])
            nc.sync.dma_start(wjbv[0][:], wjbv_d[0][:])
            nc.sync.dma_start(wjtv[0][:], wjtv_d[0][:])
            nc.sync.dma_start(sel[:], sel_d[:])
            nc.sync.dma_start(wjbv[1][:], wjbv_d[1][:])
            nc.sync.dma_start(wjtv[1][:], wjtv_d[1][:])

            fT = [packf[:, OFF_FT + c * 128 : OFF_FT + (c + 1) * 128] for c in range(4)]
            We = [packf[:, OFF_WE + c * 512 : OFF_WE + (c + 1) * 512] for c in range(4)]
            gT = [packg[:, OFF_GT + c * 64 : OFF_GT + (c + 1) * 64] for c in range(4)]
            Wd = [packg[:, OFF_WD + c * 512 : OFF_WD + (c + 1) * 512] for c in range(4)]
            be = lambda mc: packg[:, OFF_B + mc : OFF_B + mc + 1]
            bd = lambda mc: packg[:, OFF_B + 4 + mc : OFF_B + 5 + mc]
            wj_b = lambda mc, vh: wjbv[vh][:, 512 * mc : 512 * mc + 512]
            wj_t = lambda mc, vh: wjtv[vh][:, 512 * mc : 512 * mc + 512]
            ones64 = brow[:, V : V + 64]
            bj = lambda vh: brow[:, vh * 512 : (vh + 1) * 512]

            def c_gemm(ps, vh):
                """C v-half into psum partitions 64:128 (+bj)."""
                for mc in range(4):
                    nc.tensor.matmul(
                        ps[64:128, :], tgT[mc][:], wj_b(mc, vh),
                        start=(mc == 0), stop=False,
                    )
                nc.tensor.matmul(
                    ps[64:128, :], ones64, bj(vh), start=False, stop=True
                )

            def a_gemm(ps, h, vh):
                """A t-half h, v-half vh into psum partitions 0:64."""
                hs = slice(h * 64, (h + 1) * 64)
                for mc in range(4):
                    nc.tensor.matmul(
                        ps[0:64, :], tfT[mc][:, hs], wj_t(mc, vh),
                        start=(mc == 0), stop=(mc == 3),
                    )

            def c_copies(ps, vh):
                vs = slice(vh * 512, (vh + 1) * 512)
                nc.scalar.copy(ACp0[64:128, vs], ps[64:128, :])
                nc.vector.tensor_copy(ACp1[64:128, vs], ps[64:128, :])

            def a_copy(ps, h, vh):
                vs = slice(vh * 512, (vh + 1) * 512)
                acp = (ACp0, ACp1)[h]
                if h == 0:
                    nc.scalar.copy(acp[0:64, vs], ps[0:64, :])
                else:
                    nc.vector.tensor_copy(acp[0:64, vs], ps[0:64, :])

            # ---- v0 prologue ----
            with tc.tile_pool(name="pp", bufs=1, space="PSUM") as pp:
                # PE warm-up; results never read.  Interleaved with the
                # real GEMM groups so the HAM clock never idles down
                # during input-wait gaps (see module doc).
                scratch = wp.tile([128, 640], BF16, tag="scratch")
                nc.vector.memset(scratch[:], 1.0)
                wps = pp.tile([128, 512], FP32, tag="pps", bufs=4)

                def warm(n):
                    for _ in range(n):
                        nc.tensor.matmul(
                            wps[:], scratch[:, 0:128], scratch[:, 128:640],
                            start=True, stop=True,
                        )

                warm(4)
                # g-path first: C's dependency chain is longest
                for mc in range(4):
                    ms = slice(mc * 128, (mc + 1) * 128)
                    ps = pp.tile([128, U], FP32, tag="pps", bufs=4)
                    for dc in range(4):
                        nc.tensor.matmul(
                            ps[:], Wd[dc][:, ms], gT[dc],
                            start=(dc == 0), stop=(dc == 3),
                        )
                    nc.scalar.activation(tgT[mc][:], ps[:], TANH, bias=bd(mc))
                warm(3)  # bridge the wait for packf
                for mc in range(4):
                    ms = slice(mc * 128, (mc + 1) * 128)
                    ps = pp.tile([128, TC], FP32, tag="pps", bufs=4)
                    for dc in range(4):
                        nc.tensor.matmul(
                            ps[:], We[dc][:, ms], fT[dc],
                            start=(dc == 0), stop=(dc == 3),
                        )
                    nc.scalar.activation(tfT[mc][:], ps[:], TANH, bias=be(mc))
                warm(2)  # bridge the wait for wjb_v0

                psC0 = pp.tile([128, 512], FP32, tag="pj", bufs=3)
                c_gemm(psC0, 0)
                c_copies(psC0, 0)
                warm(1)  # bridge the wait for wjt_v0
                psA00 = pp.tile([128, 512], FP32, tag="pj", bufs=3)
                a_gemm(psA00, 0, 0)
                a_copy(psA00, 0, 0)
                # A-h1-v0 is NOT built here: superchunk 0 needs only
                # ACp0-v0, so it is injected after S0 (needed at J=4)

            # ---- main loop: 2 waves x 8 superchunks of [1024 rows, 512]
            # superchunk (vh, J), psum slot q: psO_q[p,:] = v-cols
            # vh*512..+512 of out flat row 1024J + 8p + q
            #   -> t = 16J + p//8, u = 8*(p%8) + q
            with (
                tc.tile_pool(name="po", bufs=1, space="PSUM") as po,
                tc.tile_pool(name="ob", bufs=5) as ob,
            ):
                def pot():
                    return po.tile([128, 1024], FP32, tag="psO", bufs=4, name="psO")

                for vh in range(2):
                    for P in range(4):
                        R0 = 512 * vh + 128 * P
                        firstp = vh == 0 and P == 0
                        lastp = vh == 1 and P == 3
                        out_sb = ob.tile([128, 8192], BF16, tag="out")
                        for jj in range(2):
                            J = 2 * P + jj
                            first = vh == 0 and J == 0
                            last = vh == 1 and J == 7
                            JJ = J % 4
                            acp = (ACp0, ACp1)[J // 4]
                            vs = slice(vh * 512, (vh + 1) * 512)
                            cb = 4096 * jj
                            for k in range(4):
                                psO = pot()
                                for i in range(2):
                                    q = 2 * k + i
                                    c0 = 128 * (8 * JJ + q)
                                    nc.tensor.matmul(
                                        psO[:, i * 512 : (i + 1) * 512],
                                        sel[:, c0 : c0 + 128],
                                        acp[:, vs],
                                        start=True, stop=True,
                                    )
                                dst = out_sb[:, cb + k * 1024 : cb + (k + 1) * 1024]
                                # steady state: ACT owns the two EARLY
                                # slots so the next superchunk's first
                                # matmul (WAR on slot 0) unblocks off
                                # ACT's first copy.  First/last
                                # superchunk alternates engines instead
                                # so its column-halves finish in
                                # parallel (serial ACT k0+k1 costs
                                # ~0.9us on the critical path there).
                                on_act = (k % 2 == 0) if (first or last) else (k < 2)
                                if on_act:
                                    nc.scalar.copy(dst, psO[:])
                                else:
                                    nc.vector.tensor_copy(dst, psO[:])
                                if (first or last) and k == 1:
                                    nc.sync.dma_start(
                                        out_d[R0 : R0 + 128, cb : cb + 2048],
                                        out_sb[:, cb : cb + 2048],
                                    )
                            if first or last:
                                nc.sync.dma_start(
                                    out_d[R0 : R0 + 128, cb + 2048 : cb + 4096],
                                    out_sb[:, cb + 2048 : cb + 4096],
                                )
                            elif firstp or lastp:
                                # partner of a split superchunk ships
                                # its own 1 MB column half
                                nc.sync.dma_start(
                                    out_d[R0 : R0 + 128, cb : cb + 4096],
                                    out_sb[:, cb : cb + 4096],
                                )

                            # v1 prologue GEMMs slot between early
                            # v0-wave superchunks once wj_v1 has landed;
                            # each injection pads to a FULL psO ring
                            # rotation so the slot phase is preserved
                            if vh == 0 and J in (0, 1, 3, 5):
                                aux = pot()[:, 0:512]
                                if J == 0:
                                    a_gemm(aux, 1, 0)
                                    a_copy(aux, 1, 0)
                                elif J == 1:
                                    c_gemm(aux, 1)
                                    c_copies(aux, 1)
                                elif J == 3:
                                    a_gemm(aux, 0, 1)
                                    a_copy(aux, 0, 1)
                                else:
                                    a_gemm(aux, 1, 1)
                                    a_copy(aux, 1, 1)
                                for _ in range(3):
                                    pot()  # phase padding, never touched
                        if not (firstp or lastp):
                            # one 2 MB DMA per pair: 1:1 [128,8192] AP,
                            # 16 KB contiguous per partition
                            nc.sync.dma_start(
                                out_d[R0 : R0 + 128, :], out_sb[:]
                            )

    nc.compile()
    return nc


def _chunkcat(M):
    """[N*128, C] -> [128, N*C]: stack 128-row chunks side by side."""
    n = M.shape[0] // 128
    return np.ascontiguousarray(
        M.reshape(n, 128, M.shape[1]).transpose(1, 0, 2).reshape(128, -1)
    )


def _build_selector():
    """Row-permuted pair selector (see main-loop comment)."""
    sel = np.zeros((128, 4096), np.float32)
    p = np.arange(128)
    for JJ in range(4):
        for q in range(8):
            col = 128 * (8 * JJ + q) + p
            sel[16 * JJ + p // 8, col] = 1.0
            sel[64 + 8 * (p % 8) + q, col] = 1.0
    return sel.astype(NPFP8)


def kernel(f, g, We, be, Wd, bd, Wj, bj):
    if "nc" not in _cache:
        _cache["nc"] = _build_nc()
    nc = _cache["nc"]

    b16 = lambda x: np.asarray(x, dtype=np.float32).astype(NPBF16)
    f = np.asarray(f, dtype=np.float32)
    g = np.asarray(g, dtype=np.float32)
    Wj = np.asarray(Wj, dtype=np.float32)

    sel = _build_selector()
    wjt = _chunkcat(b16(Wj[:D])).reshape(128, 4, 2, 512)  # [p, mc, vh, v]
    wjb = _chunkcat(b16(Wj[D:])).reshape(128, 4, 2, 512)
    brow = np.zeros((1, V + 64), np.float32)
    brow[0, :V] = np.asarray(bj, dtype=np.float32)
    brow[0, V:] = 1.0
    bias8 = np.zeros((128, 8), np.float32)
    for c in range(4):
        bias8[:, c] = np.asarray(be, dtype=np.float32)[c * 128 : (c + 1) * 128]
        bias8[:, 4 + c] = np.asarray(bd, dtype=np.float32)[c * 128 : (c + 1) * 128]
    We_p = _chunkcat(b16(We))
    Wd_p = _chunkcat(b16(Wd))

    shared = {"sel": sel, "brow": b16(brow)}
    for v in range(2):
        shared[f"wjt{v}"] = np.ascontiguousarray(wjt[:, :, v, :].reshape(128, 2048))
        shared[f"wjb{v}"] = np.ascontiguousarray(wjb[:, :, v, :].reshape(128, 2048))
    in_maps = []
    for c in range(NCORES):
        b, th = c // 2, c % 2
        fTp = _chunkcat(b16(f[b, th * TC : (th + 1) * TC, :].T))
        gTp = _chunkcat(b16(g[b].T))
        packg = np.concatenate([gTp, Wd_p, b16(bias8)], axis=1)
        packf = np.concatenate([fTp, We_p], axis=1)
        in_maps.append({
            "packg": np.ascontiguousarray(packg),
            "packf": np.ascontiguousarray(packf),
            **shared,
        })
    res = run_bass_kernel_spmd(nc, in_maps, list(range(NCORES)))
    kernel._last_results = res

    out = np.empty((B, T, U, V), np.float32)
    for c in range(NCORES):
        b, th = c // 2, c % 2
        raw = res.results[c]["out"].astype(np.float32)  # [1024, 8192]
        # unscramble pair regions: [region m, p, jj, q, v] -> flat row
        # 2048m + 1024jj + 8p + q (uniform for split and paired DMAs)
        flat = (
            raw.reshape(8, 128, 2, 8, 512)
            .transpose(0, 2, 1, 3, 4)
            .reshape(2 * 8192, 512)
        )
        half = np.empty((8192, V), np.float32)
        half[:, 0:512] = flat[0:8192]
        half[:, 512:1024] = flat[8192:16384]
        out[b, th * TC : (th + 1) * TC] = half.reshape(TC, U, V)
    return out
